# revision 2
# baseline (speedup 1.0000x reference)
"""MoE layer (8 experts, top-2) Trainium2 Bass kernel.

Strategy (expert parallelism, per sharding hint):
  - Host: replicated router math (logits -> top-2 -> softmax gates),
    dispatch = gather each expert's tokens (transposed to [D, C]).
  - Device: core e runs expert e's MLP on its gathered tokens:
        h = silu(xT.T @ W1) ; y = (h @ W2) * gate
    Matmuls run in float32r (full PE rate, ~1.5e-4 rel err).
    W1 stays resident in SBUF; W2 streams per token chunk.
  - Host: combine = scatter-add per-expert outputs into [T, D].

Fixed shapes: x [4, 2048, 1024], Wg [1024, 8], W1 [8, 1024, 4096],
W2 [8, 4096, 1024]. Tokens T = 8192. Capacity C per expert per wave is
compile-time fixed; extra waves (same NEFF) handle overflow if an
expert ever exceeds C.
"""

import sys

for _p in ("/opt/trn_rl_repo",):
    if _p not in sys.path:
        sys.path.insert(0, _p)

import numpy as np

import concourse.bass as bass  # noqa: F401  (bass types used via bacc/tile)
import concourse.mybir as mybir
import concourse.tile as tile
from concourse import bacc, bass_utils

P = 128
D = 1024
DFF = 4096
E = 8
TOPK = 2
T = 8192

KB = D // P     # 8 k-tiles over D
FB = DFF // P   # 32 tiles over DFF

CHUNK = 384     # tokens per inner chunk (N of MM1; >=256 keeps fp32r full-rate)
TT = CHUNK // P  # token tiles per chunk (3)
N_CHUNKS = 6
C = CHUNK * N_CHUNKS  # 2304 capacity per expert per wave

f32 = mybir.dt.float32
f32r = mybir.dt.float32r


def build_nc():
    nc = bacc.Bacc(None, target_bir_lowering=False)
    xT = nc.dram_tensor("xT", [D, C], f32r, kind="ExternalInput")
    w1 = nc.dram_tensor("w1", [D, DFF], f32r, kind="ExternalInput")
    w2 = nc.dram_tensor("w2", [DFF, D], f32r, kind="ExternalInput")
    g = nc.dram_tensor("g", [C, 1], f32, kind="ExternalInput")
    y = nc.dram_tensor("y", [C, D], f32, kind="ExternalOutput")

    xTr = xT.rearrange("(kb p) c -> kb p c", p=P)
    w1r = w1.rearrange("(kb p) f -> kb p f", p=P)
    w2r = w2.rearrange("(fb p) d -> fb p d", p=P)
    gr = g.rearrange("(ct p) one -> ct p one", p=P)
    yr = y.rearrange("(ct p) d -> ct p d", p=P)

    with tile.TileContext(nc) as tc:
        with (
            tc.tile_pool(name="w1pool", bufs=KB) as w1pool,
            tc.tile_pool(name="w2pool", bufs=4) as w2pool,
            tc.tile_pool(name="xpool", bufs=2 * KB) as xpool,
            tc.tile_pool(name="gpool", bufs=2 * TT) as gpool,
            tc.tile_pool(name="hpool", bufs=3) as hpool,
            tc.tile_pool(name="opool", bufs=4) as opool,
            tc.tile_pool(name="ps1pool", bufs=2, space="PSUM") as ps1pool,
            tc.tile_pool(name="ps2pool", bufs=2 * TT, space="PSUM") as ps2pool,
        ):
            # W1 resident for the whole kernel: KB tiles of [128, DFF].
            w1_tiles = []
            for kb in range(KB):
                w1t = w1pool.tile([P, DFF], f32r, tag="w1")
                nc.sync.dma_start(w1t[:], w1r[kb])
                w1_tiles.append(w1t)

            for c in range(N_CHUNKS):
                x_tiles = []
                for kb in range(KB):
                    xt_ = xpool.tile([P, CHUNK], f32r, tag="x")
                    nc.sync.dma_start(
                        xt_[:], xTr[kb, :, c * CHUNK : (c + 1) * CHUNK]
                    )
                    x_tiles.append(xt_)
                g_tiles = []
                for t in range(TT):
                    gt = gpool.tile([P, 1], f32, tag="g")
                    nc.sync.dma_start(gt[:], gr[c * TT + t])
                    g_tiles.append(gt)

                psum2 = [
                    [
                        ps2pool.tile([P, 512], f32, tag="ps2", name=f"ps2_{_t}_{_dc}")
                        for _dc in range(2)
                    ]
                    for _t in range(TT)
                ]

                # Software-pipelined over fb: MM2(fb-1) is emitted after
                # MM1(fb) so the PE never stalls on the silu between them.
                h_prev = None
                w2_prev = None
                for fb in range(FB + 1):
                    h_cur = None
                    w2_cur = None
                    if fb < FB:
                        w2_cur = w2pool.tile([P, D], f32r, tag="w2")
                        nc.sync.dma_start(w2_cur[:], w2r[fb])
                        ps1 = ps1pool.tile([P, CHUNK], f32, tag="ps1")
                        for kb in range(KB):
                            nc.tensor.matmul(
                                ps1[:],
                                w1_tiles[kb][:, fb * P : (fb + 1) * P],
                                x_tiles[kb][:],
                                start=(kb == 0),
                                stop=(kb == KB - 1),
                            )
                        h_cur = hpool.tile([P, CHUNK], f32r, tag="h")
                        nc.scalar.activation(
                            h_cur[:], ps1[:], mybir.ActivationFunctionType.Silu
                        )
                    if h_prev is not None:
                        fbp = fb - 1
                        for t in range(TT):
                            for dc in range(2):
                                nc.tensor.matmul(
                                    psum2[t][dc][:],
                                    h_prev[:, t * P : (t + 1) * P],
                                    w2_prev[:, dc * 512 : (dc + 1) * 512],
                                    start=(fbp == 0),
                                    stop=(fbp == FB - 1),
                                )
                    h_prev = h_cur
                    w2_prev = w2_cur

                for t in range(TT):
                    for dc in range(2):
                        o = opool.tile([P, 512], f32, tag="o")
                        nc.vector.tensor_scalar_mul(
                            o[:], psum2[t][dc][:], g_tiles[t][:]
                        )
                        nc.sync.dma_start(
                            yr[c * TT + t, :, dc * 512 : (dc + 1) * 512], o[:]
                        )
    nc.finalize()
    return nc


_NC_CACHE = None


def _get_nc():
    global _NC_CACHE
    if _NC_CACHE is None:
        _NC_CACHE = build_nc()
    return _NC_CACHE


def _route(xt, Wg):
    """Replicated router math in fp32 numpy: top-2 + softmax gates."""
    logits = xt @ Wg  # [T, E]
    n = logits.shape[0]
    ar = np.arange(n)
    top1 = logits.argmax(1)
    v1 = logits[ar, top1]
    masked = logits.copy()
    masked[ar, top1] = -np.inf
    top2 = masked.argmax(1)
    v2 = masked[ar, top2]
    # softmax over the two selected logits
    g1 = np.float32(1.0) / (np.float32(1.0) + np.exp(v2 - v1, dtype=np.float32))
    g2 = np.float32(1.0) - g1
    return top1, top2, g1, g2


def kernel(x, Wg, W1, W2):
    x = np.asarray(x, dtype=np.float32)
    Wg = np.asarray(Wg, dtype=np.float32)
    W1 = np.asarray(W1, dtype=np.float32)
    W2 = np.asarray(W2, dtype=np.float32)

    B, S, Dm = x.shape
    xt = x.reshape(-1, Dm)
    n_tok = xt.shape[0]

    top1, top2, g1, g2 = _route(xt, Wg)

    # Per-expert token index lists + gate values.
    sels = []
    gates = []
    for e in range(E):
        m1 = top1 == e
        m2 = top2 == e
        sel = np.flatnonzero(m1 | m2)
        gv = np.where(m1[sel], g1[sel], g2[sel]).astype(np.float32)
        sels.append(sel)
        gates.append(gv)

    nc = _get_nc()
    out = np.zeros((n_tok, Dm), dtype=np.float32)

    offs = [0] * E
    while any(offs[e] < len(sels[e]) for e in range(E)):
        in_maps = []
        wave_sel = []
        for e in range(E):
            sel = sels[e][offs[e] : offs[e] + C]
            gv = gates[e][offs[e] : offs[e] + C]
            offs[e] += len(sel)
            wave_sel.append(sel)
            xT_pad = np.zeros((D, C), dtype=np.float32)
            if len(sel):
                xT_pad[:, : len(sel)] = xt[sel].T
            g_pad = np.zeros((C, 1), dtype=np.float32)
            g_pad[: len(sel), 0] = gv
            in_maps.append(
                {"xT": xT_pad, "w1": W1[e], "w2": W2[e], "g": g_pad}
            )
        res = bass_utils.run_bass_kernel_spmd(
            nc, in_maps, core_ids=list(range(E))
        )
        for e in range(E):
            sel = wave_sel[e]
            if len(sel):
                out[sel] += res.results[e]["y"][: len(sel)]

    return out.reshape(B, S, Dm)


# revision 4
# speedup vs baseline: 1.0106x; 1.0106x over previous
"""MoE layer (8 experts, top-2) Trainium2 Bass kernel.

Strategy (expert parallelism, per sharding hint):
  - Host: replicated router math (logits -> top-2 -> softmax gates),
    dispatch = gather each expert's tokens (transposed to [D, C]).
  - Device: core e runs expert e's MLP on its gathered tokens:
        h = silu(xT.T @ W1) ; y = (h @ W2) * gate
    Matmuls run in float32r (full PE rate, ~1.5e-4 rel err).
    W1 stays resident in SBUF; W2 streams per token chunk.
  - Host: combine = scatter-add per-expert outputs into [T, D].

Fixed shapes: x [4, 2048, 1024], Wg [1024, 8], W1 [8, 1024, 4096],
W2 [8, 4096, 1024]. Tokens T = 8192. Capacity C per expert per wave is
compile-time fixed; extra waves (same NEFF) handle overflow if an
expert ever exceeds C.
"""

import sys

for _p in ("/opt/trn_rl_repo",):
    if _p not in sys.path:
        sys.path.insert(0, _p)

import numpy as np

import concourse.bass as bass  # noqa: F401  (bass types used via bacc/tile)
import concourse.mybir as mybir
import concourse.tile as tile
from concourse import bacc, bass_utils

P = 128
D = 1024
DFF = 4096
E = 8
TOPK = 2
T = 8192

KB = D // P     # 8 k-tiles over D
FB = DFF // P   # 32 tiles over DFF

CHUNK = 384     # tokens per inner chunk (N of MM1; >=256 keeps fp32r full-rate)
TT = CHUNK // P  # token tiles per chunk (3)
N_CHUNKS = 6
C = CHUNK * N_CHUNKS  # 2304 capacity per expert per wave

f32 = mybir.dt.float32
f32r = mybir.dt.float32r


def build_nc():
    nc = bacc.Bacc(None, target_bir_lowering=False)
    xT = nc.dram_tensor("xT", [D, C], f32r, kind="ExternalInput")
    w1 = nc.dram_tensor("w1", [D, DFF], f32r, kind="ExternalInput")
    w2 = nc.dram_tensor("w2", [DFF, D], f32r, kind="ExternalInput")
    g = nc.dram_tensor("g", [C, 1], f32, kind="ExternalInput")
    y = nc.dram_tensor("y", [C, D], f32, kind="ExternalOutput")

    xTr = xT.rearrange("(kb p) c -> kb p c", p=P)
    w1r = w1.rearrange("(kb p) f -> kb p f", p=P)
    w2r = w2.rearrange("(fb p) d -> fb p d", p=P)
    gr = g.rearrange("(ct p) one -> ct p one", p=P)
    yr = y.rearrange("(ct p) d -> ct p d", p=P)

    with tile.TileContext(nc) as tc:
        with (
            tc.tile_pool(name="w1pool", bufs=KB) as w1pool,
            tc.tile_pool(name="w2pool", bufs=8) as w2pool,
            tc.tile_pool(name="xpool", bufs=2 * KB) as xpool,
            tc.tile_pool(name="gpool", bufs=2 * TT) as gpool,
            tc.tile_pool(name="hpool", bufs=3) as hpool,
            tc.tile_pool(name="opool", bufs=4) as opool,
            tc.tile_pool(name="ps1pool", bufs=2, space="PSUM") as ps1pool,
            tc.tile_pool(name="ps2pool", bufs=2 * TT, space="PSUM") as ps2pool,
        ):
            # W1 resident for the whole kernel: KB tiles of [128, DFF].
            # Loaded in column segments, low-fb segments first across all
            # kb, so the first chunk's MM1s can start after ~2MB arrives
            # instead of waiting for the full 16MB.
            W1_SEG = 512
            w1_tiles = [w1pool.tile([P, DFF], f32r, tag="w1", name=f"w1_{kb}")
                        for kb in range(KB)]
            for seg in range(0, DFF, W1_SEG):
                for kb in range(KB):
                    nc.sync.dma_start(
                        w1_tiles[kb][:, seg : seg + W1_SEG],
                        w1r[kb, :, seg : seg + W1_SEG],
                    )

            for c in range(N_CHUNKS):
                x_tiles = []
                for kb in range(KB):
                    xt_ = xpool.tile([P, CHUNK], f32r, tag="x")
                    nc.sync.dma_start(
                        xt_[:], xTr[kb, :, c * CHUNK : (c + 1) * CHUNK]
                    )
                    x_tiles.append(xt_)
                g_tiles = []
                for t in range(TT):
                    gt = gpool.tile([P, 1], f32, tag="g")
                    nc.sync.dma_start(gt[:], gr[c * TT + t])
                    g_tiles.append(gt)

                psum2 = [
                    [
                        ps2pool.tile([P, 512], f32, tag="ps2", name=f"ps2_{_t}_{_dc}")
                        for _dc in range(2)
                    ]
                    for _t in range(TT)
                ]

                # Software-pipelined over fb: MM2(fb-1) is emitted after
                # MM1(fb) so the PE never stalls on the silu between them.
                h_prev = None
                w2_prev = None
                for fb in range(FB + 1):
                    h_cur = None
                    w2_cur = None
                    if fb < FB:
                        w2_cur = w2pool.tile([P, D], f32r, tag="w2")
                        nc.sync.dma_start(w2_cur[:], w2r[fb])
                        ps1 = ps1pool.tile([P, CHUNK], f32, tag="ps1")
                        for kb in range(KB):
                            nc.tensor.matmul(
                                ps1[:],
                                w1_tiles[kb][:, fb * P : (fb + 1) * P],
                                x_tiles[kb][:],
                                start=(kb == 0),
                                stop=(kb == KB - 1),
                            )
                        h_cur = hpool.tile([P, CHUNK], f32r, tag="h")
                        nc.scalar.activation(
                            h_cur[:], ps1[:], mybir.ActivationFunctionType.Silu
                        )
                    if h_prev is not None:
                        fbp = fb - 1
                        for t in range(TT):
                            for dc in range(2):
                                nc.tensor.matmul(
                                    psum2[t][dc][:],
                                    h_prev[:, t * P : (t + 1) * P],
                                    w2_prev[:, dc * 512 : (dc + 1) * 512],
                                    start=(fbp == 0),
                                    stop=(fbp == FB - 1),
                                )
                    h_prev = h_cur
                    w2_prev = w2_cur

                for t in range(TT):
                    for dc in range(2):
                        o = opool.tile([P, 512], f32, tag="o")
                        nc.vector.tensor_scalar_mul(
                            o[:], psum2[t][dc][:], g_tiles[t][:]
                        )
                        nc.sync.dma_start(
                            yr[c * TT + t, :, dc * 512 : (dc + 1) * 512], o[:]
                        )
    nc.finalize()
    return nc


_NC_CACHE = None


def _get_nc():
    global _NC_CACHE
    if _NC_CACHE is None:
        _NC_CACHE = build_nc()
    return _NC_CACHE


def _route(xt, Wg):
    """Replicated router math in fp32 numpy: top-2 + softmax gates."""
    logits = xt @ Wg  # [T, E]
    n = logits.shape[0]
    ar = np.arange(n)
    top1 = logits.argmax(1)
    v1 = logits[ar, top1]
    masked = logits.copy()
    masked[ar, top1] = -np.inf
    top2 = masked.argmax(1)
    v2 = masked[ar, top2]
    # softmax over the two selected logits
    g1 = np.float32(1.0) / (np.float32(1.0) + np.exp(v2 - v1, dtype=np.float32))
    g2 = np.float32(1.0) - g1
    return top1, top2, g1, g2


def kernel(x, Wg, W1, W2):
    x = np.asarray(x, dtype=np.float32)
    Wg = np.asarray(Wg, dtype=np.float32)
    W1 = np.asarray(W1, dtype=np.float32)
    W2 = np.asarray(W2, dtype=np.float32)

    B, S, Dm = x.shape
    xt = x.reshape(-1, Dm)
    n_tok = xt.shape[0]

    top1, top2, g1, g2 = _route(xt, Wg)

    # Per-expert token index lists + gate values.
    sels = []
    gates = []
    for e in range(E):
        m1 = top1 == e
        m2 = top2 == e
        sel = np.flatnonzero(m1 | m2)
        gv = np.where(m1[sel], g1[sel], g2[sel]).astype(np.float32)
        sels.append(sel)
        gates.append(gv)

    nc = _get_nc()
    out = np.zeros((n_tok, Dm), dtype=np.float32)

    offs = [0] * E
    while any(offs[e] < len(sels[e]) for e in range(E)):
        in_maps = []
        wave_sel = []
        for e in range(E):
            sel = sels[e][offs[e] : offs[e] + C]
            gv = gates[e][offs[e] : offs[e] + C]
            offs[e] += len(sel)
            wave_sel.append(sel)
            xT_pad = np.zeros((D, C), dtype=np.float32)
            if len(sel):
                xT_pad[:, : len(sel)] = xt[sel].T
            g_pad = np.zeros((C, 1), dtype=np.float32)
            g_pad[: len(sel), 0] = gv
            in_maps.append(
                {"xT": xT_pad, "w1": W1[e], "w2": W2[e], "g": g_pad}
            )
        res = bass_utils.run_bass_kernel_spmd(
            nc, in_maps, core_ids=list(range(E))
        )
        for e in range(E):
            sel = wave_sel[e]
            if len(sel):
                out[sel] += res.results[e]["y"][: len(sel)]

    return out.reshape(B, S, Dm)


# revision 12
# speedup vs baseline: 1.0351x; 1.0242x over previous
"""MoE layer (8 experts, top-2) Trainium2 Bass kernel.

Strategy (expert parallelism, per sharding hint):
  - Host: replicated router math (logits -> top-2 -> softmax gates),
    dispatch = gather each expert's tokens (transposed to [D, C]).
  - Device: core e runs expert e's MLP on its gathered tokens:
        h = silu(xT.T @ W1) ; y = (h @ W2) * gate
    Matmuls run in float32r (full PE rate, ~1.5e-4 rel err).
    W1 stays resident in SBUF; W2 streams per token chunk.
  - Host: combine = scatter-add per-expert outputs into [T, D].

Fixed shapes: x [4, 2048, 1024], Wg [1024, 8], W1 [8, 1024, 4096],
W2 [8, 4096, 1024]. Tokens T = 8192. Capacity C per expert per wave is
compile-time fixed; extra waves (same NEFF) handle overflow if an
expert ever exceeds C.
"""

import sys

for _p in ("/opt/trn_rl_repo",):
    if _p not in sys.path:
        sys.path.insert(0, _p)

import numpy as np

import concourse.bass as bass  # noqa: F401  (bass types used via bacc/tile)
import concourse.mybir as mybir
import concourse.tile as tile
from concourse import bacc, bass_utils

P = 128
D = 1024
DFF = 4096
E = 8
TOPK = 2
T = 8192

KB = D // P     # 8 k-tiles over D
FB = DFF // P   # 32 tiles over DFF

CHUNK = 384     # tokens per inner chunk (N of MM1; >=256 keeps fp32r full-rate)
TT = CHUNK // P  # token tiles per chunk (3)
N_CHUNKS = 6
C = CHUNK * N_CHUNKS  # 2304 capacity per expert per wave

f32 = mybir.dt.float32
f32r = mybir.dt.float32r


def build_nc():
    nc = bacc.Bacc(None, target_bir_lowering=False)
    xT = nc.dram_tensor("xT", [D, C], f32r, kind="ExternalInput")
    w1 = nc.dram_tensor("w1", [D, DFF], f32r, kind="ExternalInput")
    w2 = nc.dram_tensor("w2", [DFF, D], f32r, kind="ExternalInput")
    g = nc.dram_tensor("g", [C, 1], f32, kind="ExternalInput")
    y = nc.dram_tensor("y", [C, D], f32, kind="ExternalOutput")

    xTr = xT.rearrange("(kb p) c -> p kb c", p=P)
    w1r = w1.rearrange("(kb p) f -> kb p f", p=P)
    # two fb-tiles of W2 per DMA (halves HWDGE issue count)
    w2r = w2.rearrange("(fo two p) d -> fo p two d", two=2, p=P)
    gr = g.rearrange("(ct p) one -> p (ct one)", p=P)
    yr = y.rearrange("(ct p) d -> ct p d", p=P)

    with tile.TileContext(nc) as tc:
        with (
            tc.tile_pool(name="w1pool", bufs=KB) as w1pool,
            tc.tile_pool(name="w2pool", bufs=4) as w2pool,
            tc.tile_pool(name="xpool", bufs=2) as xpool,
            tc.tile_pool(name="gpool", bufs=2) as gpool,
            tc.tile_pool(name="hpool", bufs=3) as hpool,
            tc.tile_pool(name="opool", bufs=3) as opool,
            tc.tile_pool(name="ps1pool", bufs=2, space="PSUM") as ps1pool,
            tc.tile_pool(name="ps2pool", bufs=2 * TT, space="PSUM") as ps2pool,
        ):
            # W1 resident for the whole kernel: KB tiles of [128, DFF].
            # Segment 0 (columns for fb 0-3) loads up front on the sync
            # ring right after chunk 0's tokens; the remaining segments
            # stream on the *scalar* HWDGE ring, interleaved with chunk
            # 0's fb loop, so they never head-of-line-block the token /
            # W2 traffic on the sync ring.
            W1_SEG = 512
            SEGS_PER_FB = W1_SEG // P  # fb tiles covered per segment (4)
            w1_tiles = [w1pool.tile([P, DFF], f32r, tag="w1", name=f"w1_{kb}")
                        for kb in range(KB)]

            def load_w1_seg(s, eng):
                for kb in range(KB):
                    eng.dma_start(
                        w1_tiles[kb][:, s * W1_SEG : (s + 1) * W1_SEG],
                        w1r[kb, :, s * W1_SEG : (s + 1) * W1_SEG],
                    )

            def load_chunk_inputs(c):
                xt_ = xpool.tile([P, KB * CHUNK], f32r, tag="x", name="x_c")
                nc.sync.dma_start(
                    xt_.rearrange("p (kb c) -> p kb c", kb=KB),
                    xTr[:, :, c * CHUNK : (c + 1) * CHUNK],
                )
                x_tiles = [
                    xt_[:, kb * CHUNK : (kb + 1) * CHUNK] for kb in range(KB)
                ]
                gt = gpool.tile([P, TT], f32, tag="g", name="g_c")
                nc.sync.dma_start(gt[:], gr[:, c * TT : (c + 1) * TT])
                g_tiles = [gt[:, t : t + 1] for t in range(TT)]
                return x_tiles, g_tiles

            for c in range(N_CHUNKS):
                x_tiles, g_tiles = load_chunk_inputs(c)
                if c == 0:
                    load_w1_seg(0, nc.sync)

                psum2 = [
                    [
                        ps2pool.tile([P, 512], f32, tag="ps2", name=f"ps2_{_t}_{_dc}")
                        for _dc in range(2)
                    ]
                    for _t in range(TT)
                ]

                # Software-pipelined over fb: MM2(fb-1) is emitted after
                # MM1(fb) so the PE never stalls on the silu between them.
                h_prev = None
                w2_prev = None  # AP [P, D] for fb-1's W2 rows
                w2_pair = None
                for fb in range(FB + 1):
                    if c == 0 and fb % SEGS_PER_FB == 0:
                        s = 1 + fb // SEGS_PER_FB
                        if s < DFF // W1_SEG:
                            load_w1_seg(s, nc.scalar)
                    h_cur = None
                    w2_cur = None
                    if fb < FB:
                        if fb % 2 == 0:
                            w2_pair = w2pool.tile([P, 2 * D], f32r, tag="w2")
                            nc.sync.dma_start(
                                w2_pair.rearrange("p (two d) -> p two d", two=2),
                                w2r[fb // 2],
                            )
                        w2_cur = w2_pair[:, (fb % 2) * D : (fb % 2 + 1) * D]
                        ps1 = ps1pool.tile([P, CHUNK], f32, tag="ps1")
                        for kb in range(KB):
                            nc.tensor.matmul(
                                ps1[:],
                                w1_tiles[kb][:, fb * P : (fb + 1) * P],
                                x_tiles[kb][:],
                                start=(kb == 0),
                                stop=(kb == KB - 1),
                            )
                        h_cur = hpool.tile([P, CHUNK], f32r, tag="h")
                        nc.scalar.activation(
                            h_cur[:], ps1[:], mybir.ActivationFunctionType.Silu
                        )
                    if h_prev is not None:
                        fbp = fb - 1
                        for t in range(TT):
                            for dc in range(2):
                                nc.tensor.matmul(
                                    psum2[t][dc][:],
                                    h_prev[:, t * P : (t + 1) * P],
                                    w2_prev[:, dc * 512 : (dc + 1) * 512],
                                    start=(fbp == 0),
                                    stop=(fbp == FB - 1),
                                )
                    h_prev = h_cur
                    w2_prev = w2_cur

                for t in range(TT):
                    o = opool.tile([P, D], f32, tag="o")
                    for dc in range(2):
                        nc.vector.tensor_scalar_mul(
                            o[:, dc * 512 : (dc + 1) * 512],
                            psum2[t][dc][:],
                            g_tiles[t],
                        )
                    nc.sync.dma_start(yr[c * TT + t], o[:])
    nc.finalize()
    return nc


_NC_CACHE = None


def _get_nc():
    global _NC_CACHE
    if _NC_CACHE is None:
        _NC_CACHE = build_nc()
    return _NC_CACHE


def _route(xt, Wg):
    """Replicated router math in fp32 numpy: top-2 + softmax gates."""
    logits = xt @ Wg  # [T, E]
    n = logits.shape[0]
    ar = np.arange(n)
    top1 = logits.argmax(1)
    v1 = logits[ar, top1]
    masked = logits.copy()
    masked[ar, top1] = -np.inf
    top2 = masked.argmax(1)
    v2 = masked[ar, top2]
    # softmax over the two selected logits
    g1 = np.float32(1.0) / (np.float32(1.0) + np.exp(v2 - v1, dtype=np.float32))
    g2 = np.float32(1.0) - g1
    return top1, top2, g1, g2


def kernel(x, Wg, W1, W2):
    x = np.asarray(x, dtype=np.float32)
    Wg = np.asarray(Wg, dtype=np.float32)
    W1 = np.asarray(W1, dtype=np.float32)
    W2 = np.asarray(W2, dtype=np.float32)

    B, S, Dm = x.shape
    xt = x.reshape(-1, Dm)
    n_tok = xt.shape[0]

    top1, top2, g1, g2 = _route(xt, Wg)

    # Per-expert token index lists + gate values.
    sels = []
    gates = []
    for e in range(E):
        m1 = top1 == e
        m2 = top2 == e
        sel = np.flatnonzero(m1 | m2)
        gv = np.where(m1[sel], g1[sel], g2[sel]).astype(np.float32)
        sels.append(sel)
        gates.append(gv)

    nc = _get_nc()
    out = np.zeros((n_tok, Dm), dtype=np.float32)

    offs = [0] * E
    while any(offs[e] < len(sels[e]) for e in range(E)):
        in_maps = []
        wave_sel = []
        for e in range(E):
            sel = sels[e][offs[e] : offs[e] + C]
            gv = gates[e][offs[e] : offs[e] + C]
            offs[e] += len(sel)
            wave_sel.append(sel)
            xT_pad = np.zeros((D, C), dtype=np.float32)
            if len(sel):
                xT_pad[:, : len(sel)] = xt[sel].T
            g_pad = np.zeros((C, 1), dtype=np.float32)
            g_pad[: len(sel), 0] = gv
            in_maps.append(
                {"xT": xT_pad, "w1": W1[e], "w2": W2[e], "g": g_pad}
            )
        res = bass_utils.run_bass_kernel_spmd(
            nc, in_maps, core_ids=list(range(E))
        )
        for e in range(E):
            sel = wave_sel[e]
            if len(sel):
                out[sel] += res.results[e]["y"][: len(sel)]

    return out.reshape(B, S, Dm)


# revision 15
# speedup vs baseline: 1.0380x; 1.0028x over previous
"""MoE layer (8 experts, top-2) Trainium2 Bass kernel.

Strategy (expert parallelism, per sharding hint):
  - Host: replicated router math (logits -> top-2 -> softmax gates),
    dispatch = gather each expert's tokens (transposed to [D, C]).
  - Device: core e runs expert e's MLP on its gathered tokens:
        h = silu(xT.T @ W1) ; y = (h @ W2) * gate
    Matmuls run in float32r (full PE rate, ~1.5e-4 rel err).
    W1 stays resident in SBUF; W2 streams per token chunk.
  - Host: combine = scatter-add per-expert outputs into [T, D].

Fixed shapes: x [4, 2048, 1024], Wg [1024, 8], W1 [8, 1024, 4096],
W2 [8, 4096, 1024]. Tokens T = 8192. Capacity C per expert per wave is
compile-time fixed; extra waves (same NEFF) handle overflow if an
expert ever exceeds C.
"""

import sys

for _p in ("/opt/trn_rl_repo",):
    if _p not in sys.path:
        sys.path.insert(0, _p)

import numpy as np

import concourse.bass as bass  # noqa: F401  (bass types used via bacc/tile)
import concourse.mybir as mybir
import concourse.tile as tile
from concourse import bacc, bass_utils

P = 128
D = 1024
DFF = 4096
E = 8
TOPK = 2
T = 8192

KB = D // P     # 8 k-tiles over D
FB = DFF // P   # 32 tiles over DFF

CHUNK = 384     # tokens per inner chunk (N of MM1; >=256 keeps fp32r full-rate)
TT = CHUNK // P  # token tiles per chunk (3)
N_CHUNKS = 6
C = CHUNK * N_CHUNKS  # 2304 capacity per expert per wave

f32 = mybir.dt.float32
f32r = mybir.dt.float32r


def build_nc():
    nc = bacc.Bacc(None, target_bir_lowering=False)
    xT = nc.dram_tensor("xT", [D, C], f32r, kind="ExternalInput")
    w1 = nc.dram_tensor("w1", [D, DFF], f32r, kind="ExternalInput")
    w2 = nc.dram_tensor("w2", [DFF, D], f32r, kind="ExternalInput")
    g = nc.dram_tensor("g", [C, 1], f32, kind="ExternalInput")
    y = nc.dram_tensor("y", [C, D], f32, kind="ExternalOutput")

    xTr = xT.rearrange("(kb p) c -> p kb c", p=P)
    w1r = w1.rearrange("(kb p) f -> p kb f", p=P)
    # two fb-tiles of W2 per DMA (halves HWDGE issue count)
    w2r = w2.rearrange("(fo two p) d -> fo p two d", two=2, p=P)
    gr = g.rearrange("(ct p) one -> p (ct one)", p=P)
    yr = y.rearrange("(ct p) d -> ct p d", p=P)

    with tile.TileContext(nc) as tc:
        with (
            tc.tile_pool(name="w1pool", bufs=1) as w1pool,
            tc.tile_pool(name="w2pool", bufs=4) as w2pool,
            tc.tile_pool(name="xpool", bufs=2) as xpool,
            tc.tile_pool(name="gpool", bufs=2) as gpool,
            tc.tile_pool(name="hpool", bufs=3) as hpool,
            tc.tile_pool(name="opool", bufs=3) as opool,
            tc.tile_pool(name="ps1pool", bufs=2, space="PSUM") as ps1pool,
            tc.tile_pool(name="ps2pool", bufs=2 * TT, space="PSUM") as ps2pool,
        ):
            # W1 resident for the whole kernel: KB tiles of [128, DFF].
            # Segment 0 (columns for fb 0-3) loads up front on the sync
            # ring right after chunk 0's tokens; the remaining segments
            # stream on the *scalar* HWDGE ring, interleaved with chunk
            # 0's fb loop, so they never head-of-line-block the token /
            # W2 traffic on the sync ring.
            W1_SEG = 512
            SEGS_PER_FB = W1_SEG // P  # fb tiles covered per segment (4)
            w1sb = w1pool.tile([P, KB * DFF], f32r, tag="w1", name="w1sb")
            w1v = w1sb.rearrange("p (kb f) -> p kb f", kb=KB)
            w1_tiles = [w1sb[:, kb * DFF : (kb + 1) * DFF] for kb in range(KB)]

            def load_w1_seg(s, eng):
                eng.dma_start(
                    w1v[:, :, s * W1_SEG : (s + 1) * W1_SEG],
                    w1r[:, :, s * W1_SEG : (s + 1) * W1_SEG],
                )

            def load_chunk_inputs(c):
                xt_ = xpool.tile([P, KB * CHUNK], f32r, tag="x", name="x_c")
                nc.sync.dma_start(
                    xt_.rearrange("p (kb c) -> p kb c", kb=KB),
                    xTr[:, :, c * CHUNK : (c + 1) * CHUNK],
                )
                x_tiles = [
                    xt_[:, kb * CHUNK : (kb + 1) * CHUNK] for kb in range(KB)
                ]
                gt = gpool.tile([P, TT], f32, tag="g", name="g_c")
                nc.sync.dma_start(gt[:], gr[:, c * TT : (c + 1) * TT])
                g_tiles = [gt[:, t : t + 1] for t in range(TT)]
                return x_tiles, g_tiles

            for c in range(N_CHUNKS):
                x_tiles, g_tiles = load_chunk_inputs(c)
                if c == 0:
                    load_w1_seg(0, nc.sync)

                psum2 = [
                    [
                        ps2pool.tile([P, 512], f32, tag="ps2", name=f"ps2_{_t}_{_dc}")
                        for _dc in range(2)
                    ]
                    for _t in range(TT)
                ]

                # Software-pipelined over fb: MM2(fb-1) is emitted after
                # MM1(fb) so the PE never stalls on the silu between them.
                h_prev = None
                w2_prev = None  # AP [P, D] for fb-1's W2 rows
                w2_pair = None
                for fb in range(FB + 1):
                    if c == 0 and fb % SEGS_PER_FB == 0:
                        s = 1 + fb // SEGS_PER_FB
                        if s < DFF // W1_SEG:
                            load_w1_seg(s, nc.scalar)
                    h_cur = None
                    w2_cur = None
                    if fb < FB:
                        if fb % 2 == 0:
                            w2_pair = w2pool.tile([P, 2 * D], f32r, tag="w2")
                            nc.sync.dma_start(
                                w2_pair.rearrange("p (two d) -> p two d", two=2),
                                w2r[fb // 2],
                            )
                        w2_cur = w2_pair[:, (fb % 2) * D : (fb % 2 + 1) * D]
                        ps1 = ps1pool.tile([P, CHUNK], f32, tag="ps1")
                        for kb in range(KB):
                            nc.tensor.matmul(
                                ps1[:],
                                w1_tiles[kb][:, fb * P : (fb + 1) * P],
                                x_tiles[kb][:],
                                start=(kb == 0),
                                stop=(kb == KB - 1),
                            )
                        h_cur = hpool.tile([P, CHUNK], f32r, tag="h")
                        nc.scalar.activation(
                            h_cur[:], ps1[:], mybir.ActivationFunctionType.Silu
                        )
                    if h_prev is not None:
                        fbp = fb - 1
                        for t in range(TT):
                            for dc in range(2):
                                nc.tensor.matmul(
                                    psum2[t][dc][:],
                                    h_prev[:, t * P : (t + 1) * P],
                                    w2_prev[:, dc * 512 : (dc + 1) * 512],
                                    start=(fbp == 0),
                                    stop=(fbp == FB - 1),
                                )
                    h_prev = h_cur
                    w2_prev = w2_cur

                for t in range(TT):
                    o = opool.tile([P, D], f32, tag="o")
                    for dc in range(2):
                        nc.vector.tensor_scalar_mul(
                            o[:, dc * 512 : (dc + 1) * 512],
                            psum2[t][dc][:],
                            g_tiles[t],
                        )
                    nc.sync.dma_start(yr[c * TT + t], o[:])
    nc.finalize()
    return nc


_NC_CACHE = None


def _get_nc():
    global _NC_CACHE
    if _NC_CACHE is None:
        _NC_CACHE = build_nc()
    return _NC_CACHE


def _route(xt, Wg):
    """Replicated router math in fp32 numpy: top-2 + softmax gates."""
    logits = xt @ Wg  # [T, E]
    n = logits.shape[0]
    ar = np.arange(n)
    top1 = logits.argmax(1)
    v1 = logits[ar, top1]
    masked = logits.copy()
    masked[ar, top1] = -np.inf
    top2 = masked.argmax(1)
    v2 = masked[ar, top2]
    # softmax over the two selected logits
    g1 = np.float32(1.0) / (np.float32(1.0) + np.exp(v2 - v1, dtype=np.float32))
    g2 = np.float32(1.0) - g1
    return top1, top2, g1, g2


def kernel(x, Wg, W1, W2):
    x = np.asarray(x, dtype=np.float32)
    Wg = np.asarray(Wg, dtype=np.float32)
    W1 = np.asarray(W1, dtype=np.float32)
    W2 = np.asarray(W2, dtype=np.float32)

    B, S, Dm = x.shape
    xt = x.reshape(-1, Dm)
    n_tok = xt.shape[0]

    top1, top2, g1, g2 = _route(xt, Wg)

    # Per-expert token index lists + gate values.
    sels = []
    gates = []
    for e in range(E):
        m1 = top1 == e
        m2 = top2 == e
        sel = np.flatnonzero(m1 | m2)
        gv = np.where(m1[sel], g1[sel], g2[sel]).astype(np.float32)
        sels.append(sel)
        gates.append(gv)

    nc = _get_nc()
    out = np.zeros((n_tok, Dm), dtype=np.float32)

    offs = [0] * E
    while any(offs[e] < len(sels[e]) for e in range(E)):
        in_maps = []
        wave_sel = []
        for e in range(E):
            sel = sels[e][offs[e] : offs[e] + C]
            gv = gates[e][offs[e] : offs[e] + C]
            offs[e] += len(sel)
            wave_sel.append(sel)
            xT_pad = np.zeros((D, C), dtype=np.float32)
            if len(sel):
                xT_pad[:, : len(sel)] = xt[sel].T
            g_pad = np.zeros((C, 1), dtype=np.float32)
            g_pad[: len(sel), 0] = gv
            in_maps.append(
                {"xT": xT_pad, "w1": W1[e], "w2": W2[e], "g": g_pad}
            )
        res = bass_utils.run_bass_kernel_spmd(
            nc, in_maps, core_ids=list(range(E))
        )
        for e in range(E):
            sel = wave_sel[e]
            if len(sel):
                out[sel] += res.results[e]["y"][: len(sel)]

    return out.reshape(B, S, Dm)


# revision 19
# speedup vs baseline: 1.0620x; 1.0232x over previous
"""MoE layer (8 experts, top-2) Trainium2 Bass kernel.

Strategy (expert parallelism, per sharding hint):
  - Host: replicated router math (logits -> top-2 -> softmax gates),
    dispatch = gather each expert's tokens (transposed to [D, C]).
  - Device: core e runs expert e's MLP on its gathered tokens:
        h = silu(xT.T @ W1) ; y = (h @ W2) * gate
    Matmuls run in float32r (full PE rate, ~1.5e-4 rel err).
    W1 stays resident in SBUF; W2 streams per token chunk.
  - Host: combine = scatter-add per-expert outputs into [T, D].

Fixed shapes: x [4, 2048, 1024], Wg [1024, 8], W1 [8, 1024, 4096],
W2 [8, 4096, 1024]. Tokens T = 8192. Capacity C per expert per wave is
compile-time fixed; extra waves (same NEFF) handle overflow if an
expert ever exceeds C.
"""

import sys

for _p in ("/opt/trn_rl_repo",):
    if _p not in sys.path:
        sys.path.insert(0, _p)

import numpy as np

import concourse.bass as bass  # noqa: F401  (bass types used via bacc/tile)
import concourse.mybir as mybir
import concourse.tile as tile
from concourse import bacc, bass_utils

P = 128
D = 1024
DFF = 4096
E = 8
TOPK = 2
T = 8192

KB = D // P     # 8 k-tiles over D
FB = DFF // P   # 32 tiles over DFF

CHUNK = 384     # tokens per inner chunk (N of MM1; >=256 keeps fp32r full-rate)
TT = CHUNK // P  # token tiles per chunk (3)
N_CHUNKS = 6
C = CHUNK * N_CHUNKS  # 2304 capacity per expert per wave

f32 = mybir.dt.float32
f32r = mybir.dt.float32r


def build_nc():
    nc = bacc.Bacc(None, target_bir_lowering=False)
    xT = nc.dram_tensor("xT", [D, C], f32r, kind="ExternalInput")
    w1 = nc.dram_tensor("w1", [D, DFF], f32r, kind="ExternalInput")
    w2 = nc.dram_tensor("w2", [DFF, D], f32r, kind="ExternalInput")
    g = nc.dram_tensor("g", [C, 1], f32, kind="ExternalInput")
    y = nc.dram_tensor("y", [C, D], f32, kind="ExternalOutput")

    xTr = xT.rearrange("(kb p) c -> p kb c", p=P)
    w1r = w1.rearrange("(kb p) f -> p kb f", p=P)
    # two fb-tiles of W2 per DMA (halves HWDGE issue count)
    w2r = w2.rearrange("(fo two p) d -> fo p two d", two=2, p=P)
    gr = g.rearrange("(ct p) one -> p (ct one)", p=P)
    yr = y.rearrange("(ct p) d -> ct p d", p=P)

    with tile.TileContext(nc) as tc:
        with (
            tc.tile_pool(name="w1pool", bufs=1) as w1pool,
            tc.tile_pool(name="w2pool", bufs=4) as w2pool,
            tc.tile_pool(name="xpool", bufs=2) as xpool,
            tc.tile_pool(name="gpool", bufs=2) as gpool,
            tc.tile_pool(name="hpool", bufs=4) as hpool,
            tc.tile_pool(name="opool", bufs=3) as opool,
            tc.tile_pool(name="ps1pool", bufs=2, space="PSUM") as ps1pool,
            tc.tile_pool(name="ps2pool", bufs=2 * TT, space="PSUM") as ps2pool,
        ):
            # W1 resident for the whole kernel: KB tiles of [128, DFF].
            # Segment 0 (columns for fb 0-3) loads up front on the sync
            # ring right after chunk 0's tokens; the remaining segments
            # stream on the *scalar* HWDGE ring, interleaved with chunk
            # 0's fb loop, so they never head-of-line-block the token /
            # W2 traffic on the sync ring.
            W1_SEG = 512
            SEGS_PER_FB = W1_SEG // P  # fb tiles covered per segment (4)
            w1sb = w1pool.tile([P, KB * DFF], f32r, tag="w1", name="w1sb")
            w1v = w1sb.rearrange("p (kb f) -> p kb f", kb=KB)
            w1_tiles = [w1sb[:, kb * DFF : (kb + 1) * DFF] for kb in range(KB)]

            def load_w1_cols(eng, lo, hi):
                eng.dma_start(w1v[:, :, lo:hi], w1r[:, :, lo:hi])

            def load_chunk_inputs(c):
                xt_ = xpool.tile([P, KB * CHUNK], f32r, tag="x", name="x_c")
                nc.sync.dma_start(
                    xt_.rearrange("p (kb c) -> p kb c", kb=KB),
                    xTr[:, :, c * CHUNK : (c + 1) * CHUNK],
                )
                x_tiles = [
                    xt_[:, kb * CHUNK : (kb + 1) * CHUNK] for kb in range(KB)
                ]
                gt = gpool.tile([P, TT], f32, tag="g", name="g_c")
                nc.sync.dma_start(gt[:], gr[:, c * TT : (c + 1) * TT])
                g_tiles = [gt[:, t : t + 1] for t in range(TT)]
                return x_tiles, g_tiles

            for c in range(N_CHUNKS):
                x_tiles, g_tiles = load_chunk_inputs(c)
                if c == 0:
                    # fb 0-1 / 2-3 columns first, on the otherwise-idle
                    # scalar ring, so MM1 starts as soon as x lands.
                    load_w1_cols(nc.scalar, 0, 2 * P)
                    load_w1_cols(nc.scalar, 2 * P, W1_SEG)

                psum2 = [
                    [
                        ps2pool.tile([P, 512], f32, tag="ps2", name=f"ps2_{_t}_{_dc}")
                        for _dc in range(2)
                    ]
                    for _t in range(TT)
                ]

                # Software-pipelined over fb: MM2(fb-1) is emitted after
                # MM1(fb) so the PE never stalls on the silu between them.
                h_prev = None
                w2_prev = None  # AP [P, D] for fb-1's W2 rows
                w2_pair = None
                for fb in range(FB + 1):
                    if c == 0 and fb % SEGS_PER_FB == 0:
                        s = 1 + fb // SEGS_PER_FB
                        if s < DFF // W1_SEG:
                            load_w1_cols(nc.scalar, s * W1_SEG, (s + 1) * W1_SEG)
                    h_cur = None
                    w2_cur = None
                    if fb < FB:
                        if fb % 2 == 0:
                            w2_pair = w2pool.tile([P, 2 * D], f32r, tag="w2")
                            nc.sync.dma_start(
                                w2_pair.rearrange("p (two d) -> p two d", two=2),
                                w2r[fb // 2],
                            )
                        w2_cur = w2_pair[:, (fb % 2) * D : (fb % 2 + 1) * D]
                        ps1 = ps1pool.tile([P, CHUNK], f32, tag="ps1")
                        for kb in range(KB):
                            nc.tensor.matmul(
                                ps1[:],
                                w1_tiles[kb][:, fb * P : (fb + 1) * P],
                                x_tiles[kb][:],
                                start=(kb == 0),
                                stop=(kb == KB - 1),
                            )
                        h_cur = hpool.tile([P, CHUNK], f32r, tag="h")
                        nc.scalar.activation(
                            h_cur[:], ps1[:], mybir.ActivationFunctionType.Silu
                        )
                    if h_prev is not None:
                        fbp = fb - 1
                        for t in range(TT):
                            for dc in range(2):
                                nc.tensor.matmul(
                                    psum2[t][dc][:],
                                    h_prev[:, t * P : (t + 1) * P],
                                    w2_prev[:, dc * 512 : (dc + 1) * 512],
                                    start=(fbp == 0),
                                    stop=(fbp == FB - 1),
                                )
                    h_prev = h_cur
                    w2_prev = w2_cur

                for t in range(TT):
                    o = opool.tile([P, D], f32, tag="o")
                    for dc in range(2):
                        nc.vector.tensor_scalar_mul(
                            o[:, dc * 512 : (dc + 1) * 512],
                            psum2[t][dc][:],
                            g_tiles[t],
                        )
                    nc.sync.dma_start(yr[c * TT + t], o[:])
    nc.finalize()
    return nc


_NC_CACHE = None


def _get_nc():
    global _NC_CACHE
    if _NC_CACHE is None:
        _NC_CACHE = build_nc()
    return _NC_CACHE


def _route(xt, Wg):
    """Replicated router math in fp32 numpy: top-2 + softmax gates."""
    logits = xt @ Wg  # [T, E]
    n = logits.shape[0]
    ar = np.arange(n)
    top1 = logits.argmax(1)
    v1 = logits[ar, top1]
    masked = logits.copy()
    masked[ar, top1] = -np.inf
    top2 = masked.argmax(1)
    v2 = masked[ar, top2]
    # softmax over the two selected logits
    g1 = np.float32(1.0) / (np.float32(1.0) + np.exp(v2 - v1, dtype=np.float32))
    g2 = np.float32(1.0) - g1
    return top1, top2, g1, g2


def kernel(x, Wg, W1, W2):
    x = np.asarray(x, dtype=np.float32)
    Wg = np.asarray(Wg, dtype=np.float32)
    W1 = np.asarray(W1, dtype=np.float32)
    W2 = np.asarray(W2, dtype=np.float32)

    B, S, Dm = x.shape
    xt = x.reshape(-1, Dm)
    n_tok = xt.shape[0]

    top1, top2, g1, g2 = _route(xt, Wg)

    # Per-expert token index lists + gate values.
    sels = []
    gates = []
    for e in range(E):
        m1 = top1 == e
        m2 = top2 == e
        sel = np.flatnonzero(m1 | m2)
        gv = np.where(m1[sel], g1[sel], g2[sel]).astype(np.float32)
        sels.append(sel)
        gates.append(gv)

    nc = _get_nc()
    out = np.zeros((n_tok, Dm), dtype=np.float32)

    offs = [0] * E
    while any(offs[e] < len(sels[e]) for e in range(E)):
        in_maps = []
        wave_sel = []
        for e in range(E):
            sel = sels[e][offs[e] : offs[e] + C]
            gv = gates[e][offs[e] : offs[e] + C]
            offs[e] += len(sel)
            wave_sel.append(sel)
            xT_pad = np.zeros((D, C), dtype=np.float32)
            if len(sel):
                xT_pad[:, : len(sel)] = xt[sel].T
            g_pad = np.zeros((C, 1), dtype=np.float32)
            g_pad[: len(sel), 0] = gv
            in_maps.append(
                {"xT": xT_pad, "w1": W1[e], "w2": W2[e], "g": g_pad}
            )
        res = bass_utils.run_bass_kernel_spmd(
            nc, in_maps, core_ids=list(range(E))
        )
        for e in range(E):
            sel = wave_sel[e]
            if len(sel):
                out[sel] += res.results[e]["y"][: len(sel)]

    return out.reshape(B, S, Dm)
